# revision 26
# baseline (speedup 1.0000x reference)
"""GCN encoder (2-layer GCN with shared graph) on 8 Trainium2 NeuronCores.

Math (per gcn_conv, PyG GCNConv with edge weights, self-loops in edge list):
    deg[v]  = sum of w over edges (s -> v)            (in-degree, weighted)
    dinv    = deg ** -0.5                             (deg >= 1 always: self-loops)
    out[d]  = dinv[d] * sum_s Wgt[s,d] * dinv[s] * h[s] @ W + b
where Wgt[s,d] = count(edge_index) + I (self loops) + sigmoid(masked_y[:1024,:1024])
          (the sigmoid part only on the [0:1024) x [0:1024) block)

Design: single-collective.  Measurement shows the FIRST collective of a NEFF
execution completes at ~(last core start + 68us) regardless of payload or
trigger time (host-mediated CC-channel bring-up), while later collectives are
fast.  So the kernel uses exactly ONE collective (the unavoidable hidden-state
exchange) and puts everything else before that wall:

  - each core holds: its 256-dst-column adjacency shard (host-counted ints),
    the FULL masked_y^T (2MB bf16, replicated), the FULL x, and the host's
    integer degree counts (replicated).
  - full degree vector is computed redundantly per core: sigmoid(masked_y^T)
    on ACT + free-dim reduce on DVE gives deg_sig in per-partition layout
    [128,8]; + count degrees -> dinv for ALL 2048 nodes, no exchange.
  - dinv[src] is folded into the adjacency rows (adj~ = dinv_s * Wgt); layer 1
    runs fully locally; ONE AllGather exchanges hidden [2048,128] bf16;
    layer 2 runs locally.
  - second-stage matmuls use agg chunks as stationary and W as moving, which
    yields [dst, feat] outputs directly - no PE transposes anywhere.
    out = dinv_d*(agg@W + sqrt(deg_d)*b) with relu(dinv*x)=dinv*relu(x).

All float math (sigmoid, degrees, normalization, aggregation, dense layers)
runs on device; the host only counts integer edges and lays out memory.
"""

import numpy as np

N = 2048
HALF = 1024
F = 128          # IN_C == HID == 128
NCORES = 8
NT = 16          # 16 src-row tiles of 128
CPC = 256        # columns (dst nodes) per core

USE_BF16 = True

_COMPILED = {}


def _np_dt(use_bf16):
    if use_bf16:
        import ml_dtypes
        return np.dtype(ml_dtypes.bfloat16)
    return np.dtype(np.float32)


def _build_program(use_bf16):
    import concourse.bacc as bacc
    import concourse.tile as tile
    from concourse import mybir

    f32 = mybir.dt.float32
    DT = mybir.dt.bfloat16 if use_bf16 else f32
    FP8 = mybir.dt.float8e4
    npdt = _np_dt(use_bf16)
    AF = mybir.ActivationFunctionType
    MUL = mybir.AluOpType.mult
    ADD = mybir.AluOpType.add
    AX = mybir.AxisListType
    # layer 2 runs in fp8 (hidden is post-relu positive; aggregation sums
    # ~1e3 positive terms so fp8 rounding averages out).  Scales keep the
    # small adjacency/hidden values out of the fp8 subnormal range; the
    # combined factor divides back out of the final per-partition scale.
    S_ADJ, S_H = 32.0, 32.0
    S_L2 = S_ADJ * S_H

    nc = bacc.Bacc(
        "TRN2",
        target_bir_lowering=False,
        debug=False,
        enable_asserts=False,
        num_devices=NCORES,
    )

    # I/O (layouts pre-swizzled on host to [128, ...])
    adj_d = nc.dram_tensor("adj", [128, NT * CPC], DT, kind="ExternalInput")
    my_d = nc.dram_tensor("my", [128, 8 * F], DT, kind="ExternalInput")
    myt_d = nc.dram_tensor("myt", [128, 8 * HALF], DT, kind="ExternalInput")
    xf_d = nc.dram_tensor("xf", [128, NT * F], DT, kind="ExternalInput")
    cdeg_d = nc.dram_tensor("cdeg", [128, NT], f32, kind="ExternalInput")
    w1_d = nc.dram_tensor("w1", [F, F], DT, kind="ExternalInput")
    w2_d = nc.dram_tensor("w2", [F, F], DT, kind="ExternalInput")
    b1_d = nc.dram_tensor("b1", [1, F], DT, kind="ExternalInput")
    b2_d = nc.dram_tensor("b2", [1, F], DT, kind="ExternalInput")
    z_d = nc.dram_tensor("z", [128, 2 * F], f32, kind="ExternalOutput")

    ones_col_d = nc.inline_tensor(np.ones((128, 1), npdt), "ones_col")
    ones11_d = nc.inline_tensor(np.ones((1, 1), np.float32), "ones11")

    rg = [list(range(NCORES))]

    with tile.TileContext(nc) as tc:
        with (
            tc.tile_pool(name="big", bufs=1) as big,
            tc.tile_pool(name="work", bufs=2) as work,
            tc.tile_pool(name="ps", bufs=1, space="PSUM") as ps,
            tc.tile_pool(name="dram", bufs=1, space="DRAM") as dram,
        ):
            # ---- loads ----
            # scalar issues nothing: DMA issues serialize with ACT compute
            # on the same engine and would stall the sigmoid chain.
            onec = big.tile([128, 1], DT, name="onec_sb")
            nc.gpsimd.dma_start(onec[:], ones_col_d.ap())
            ones11 = big.tile([1, 1], f32, name="ones11_sb")
            nc.gpsimd.dma_start(ones11[:], ones11_d.ap())
            cdeg = big.tile([128, NT], f32, name="cdeg_sb")
            nc.gpsimd.dma_start(cdeg[:], cdeg_d.ap())
            myt = big.tile([128, 8 * HALF], DT, name="myt_sb")
            for j in range(4):
                nc.sync.dma_start(myt[:, 2 * HALF * j:2 * HALF * (j + 1)],
                                  myt_d.ap()[:, 2 * HALF * j:2 * HALF * (j + 1)])
            myb = big.tile([128, 8 * F], DT, name="my_sb")
            nc.gpsimd.dma_start(myb[:], my_d.ap())
            adj = big.tile([128, NT * CPC], DT, name="adj_sb")
            # tiles 8-15 first (degree matmuls need them before the
            # sigmoid-gated tiles 0-7)
            nc.gpsimd.dma_start(adj[:, 8 * CPC:], adj_d.ap()[:, 8 * CPC:])
            nc.gpsimd.dma_start(adj[:, :8 * CPC], adj_d.ap()[:, :8 * CPC])
            xf = big.tile([128, NT * F], DT, name="xf_sb")
            nc.gpsimd.dma_start(xf[:], xf_d.ap())
            w1s = big.tile([F, F], DT, name="w1_sb")
            nc.gpsimd.dma_start(w1s[:], w1_d.ap())
            w2s = big.tile([F, F], DT, name="w2_sb")
            nc.gpsimd.dma_start(w2s[:], w2_d.ap())
            b1s = big.tile([1, F], DT, name="b1_sb")
            nc.gpsimd.dma_start(b1s[:], b1_d.ap())
            b2s = big.tile([1, F], DT, name="b2_sb")
            nc.gpsimd.dma_start(b2s[:], b2_d.ap())

            # ---- full deg_sig: sigmoid(masked_y^T) + free-dim reduce ----
            # degsig[p, j] = sum_s sigmoid(masked_y[s, 128j+p])
            degsig = big.tile([128, 8], f32, name="degsig_sb")
            for j in range(4):
                sgt = work.tile([128, 2 * HALF], DT, tag="sgt")
                nc.scalar.activation(sgt[:], myt[:, 2 * HALF * j:2 * HALF * (j + 1)],
                                     AF.Sigmoid)
                for u in range(2):
                    nc.vector.tensor_reduce(
                        degsig[:, 2 * j + u:2 * j + u + 1],
                        sgt[:, HALF * u:HALF * (u + 1)], axis=AX.X, op=ADD)

            # ---- Wgt dense block: adj += sigmoid(masked_y own-col shard) ----
            sg = work.tile([128, 8 * F], DT, tag="sg")
            nc.scalar.activation(sg[:], myb[:], AF.Sigmoid)
            for i in range(8):
                nc.vector.tensor_add(
                    adj[:, CPC * i:CPC * i + F], adj[:, CPC * i:CPC * i + F],
                    sg[:, F * i:F * (i + 1)])

            # ---- full dinv in per-partition layout [128, 16] ----
            dfull = big.tile([128, 8], f32, name="dfull_sb")
            nc.vector.tensor_tensor(dfull[:], cdeg[:, 0:8], degsig[:], op=ADD)
            sq_bc = big.tile([128, NT], f32, name="sqbc_sb")
            nc.scalar.activation(sq_bc[:, 0:8], dfull[:], AF.Sqrt)
            nc.scalar.activation(sq_bc[:, 8:16], cdeg[:, 8:16], AF.Sqrt)
            dinv_bc = big.tile([128, NT], f32, name="dinvbc_sb")
            nc.vector.reciprocal(dinv_bc[:], sq_bc[:])

            # ---- own-column degree row (for bias trick + dco) ----
            # colsums of the full Wgt shard; tiles 8-15 first (no sigmoid dep)
            ps_deg = ps.tile([1, CPC], f32, name="ps_deg")
            t_order = list(range(8, NT)) + list(range(8))
            for i, t in enumerate(t_order):
                nc.tensor.matmul(
                    ps_deg[:], onec[:], adj[:, CPC * t:CPC * (t + 1)],
                    start=(i == 0), stop=(i == NT - 1),
                )
            sqd = big.tile([1, CPC], f32, name="sqd_sb")     # sqrt(deg) (own)
            nc.scalar.activation(sqd[:], ps_deg[:], AF.Sqrt)
            sqb = big.tile([1, CPC], DT, name="sqb_sb")      # bf16 for bias mm
            nc.vector.tensor_copy(sqb[:], sqd[:])
            # dco[p, h] = dinv[own node 128h+p] via tiny transpose matmuls
            ps_dc = ps.tile([128, 2], f32, name="ps_dc")
            for h in range(2):
                nc.tensor.matmul(ps_dc[:, h:h + 1],
                                 sqd[:, 128 * h:128 * (h + 1)],
                                 ones11[:], start=(h == 0), stop=(h == 1))
            dco = big.tile([128, 2], f32, name="dco_sb")
            nc.vector.reciprocal(dco[:], ps_dc[:])

            # ---- fold dinv[src] into adjacency rows: adj~ = dinv_s * Wgt ----
            for t in range(NT):
                nc.vector.tensor_scalar_mul(
                    adj[:, CPC * t:CPC * (t + 1)], adj[:, CPC * t:CPC * (t + 1)],
                    dinv_bc[:, t:t + 1],
                )
            # fp8 copy of adj~ for layer 2
            adj8 = big.tile([128, NT * CPC], FP8, name="adj8_sb")
            nc.vector.tensor_scalar_mul(adj8[:], adj[:], S_ADJ)
            dco8 = big.tile([128, 2], f32, name="dco8_sb")
            nc.vector.tensor_scalar_mul(dco8[:], dco[:], S_H)
            dcoz = big.tile([128, 2], f32, name="dcoz_sb")
            nc.vector.tensor_scalar_mul(dcoz[:], dco[:], 1.0 / S_L2)
            sqbl2 = big.tile([1, CPC], DT, name="sqbl2_sb")
            nc.vector.tensor_scalar_mul(sqbl2[:], sqd[:], S_L2)

            def layer(xtiles, adjt, wsb, bsb, sqrow, name):
                # aggT[f, d] = sum_s x[s, f] * adj~[s, d]
                ps_agg = ps.tile([128, CPC], f32, name=f"ps_agg_{name}",
                                 tag="ps_agg")
                for t in range(NT):
                    nc.tensor.matmul(
                        ps_agg[:], xtiles[:, F * t:F * (t + 1)],
                        adjt[:, CPC * t:CPC * (t + 1)],
                        start=(t == 0), stop=(t == NT - 1),
                    )
                # r[d, o] = sum_f aggT[f, d]*W[f, o] + sqrt(deg_d)*b[o]
                # (agg chunk as stationary -> [dst, feat] directly); the
                # PSUM->SBUF cast is split per half so the first half's W
                # matmul overlaps the second half's cast.
                aggs = work.tile([128, CPC], DT, tag="aggs")
                ps_r = ps.tile([128, CPC], f32, name=f"ps_r_{name}", tag="ps_r")
                for h in range(2):
                    nc.vector.tensor_copy(aggs[:, 128 * h:128 * (h + 1)],
                                          ps_agg[:, 128 * h:128 * (h + 1)])
                    nc.tensor.matmul(ps_r[:, 128 * h:128 * (h + 1)],
                                     aggs[:, 128 * h:128 * (h + 1)], wsb[:],
                                     start=True, stop=False)
                    nc.tensor.matmul(ps_r[:, 128 * h:128 * (h + 1)],
                                     sqrow[:, 128 * h:128 * (h + 1)], bsb[:],
                                     start=False, stop=True)
                return ps_r

            # ---- layer 1 (fully local) ----
            ps_r1 = layer(xf, adj, w1s, b1s, sqb, "l1")
            # hidden[d,o] = relu(dinv_d*r) = dinv_d*relu(r)   (dinv > 0)
            g01 = work.tile([128, 2 * F], FP8, tag="g01")
            rl1 = work.tile([128, 2 * F], DT, tag="rl1")
            for h in range(2):
                nc.scalar.activation(rl1[:, 128 * h:128 * (h + 1)],
                                     ps_r1[:, 128 * h:128 * (h + 1)], AF.Relu)
                nc.vector.tensor_scalar_mul(
                    g01[:, 128 * h:128 * (h + 1)],
                    rl1[:, 128 * h:128 * (h + 1)], dco8[:, h:h + 1])

            # ---- the single collective: gather hidden [2048, 128] fp8 ----
            ag_in = dram.tile([CPC, F], FP8, name="ag_in")
            ag_out = dram.tile([N, F], FP8, name="ag_out", addr_space="Shared")
            nc.scalar.dma_start(ag_in[:].rearrange("(h p) c -> p h c", h=2), g01[:])
            nc.gpsimd.collective_compute(
                "AllGather", mybir.AluOpType.bypass,
                replica_groups=rg, ins=[ag_in.opt()], outs=[ag_out.opt()],
            )

            # gathered row 256k+128h+p = hidden[node 1024h+128k+p]; src tile
            # t=8h+k is a contiguous [128,128] block -> 3 parallel reloads
            xb2 = big.tile([128, NT * F], FP8, name="xb2_sb")
            v = ag_out[:].rearrange("(k h p) c -> h p k c", k=8, h=2)
            # one issue per queue, smallest first so matmul t=0 starts
            # while the rest still streams
            nc.scalar.dma_start(xb2[:, 0:2 * F], v[0][:, 0:2])
            nc.sync.dma_start(xb2[:, 2 * F:8 * F], v[0][:, 2:8])
            nc.gpsimd.dma_start(xb2[:, 8 * F:16 * F], v[1])

            # ---- layer 2 ----
            ps_r2 = layer(xb2, adj8, w2s, b2s, sqbl2, "l2")
            zt = work.tile([128, 2 * F], f32, tag="zt")
            zq = [nc.scalar, nc.sync]
            for h in range(2):
                nc.vector.tensor_scalar_mul(
                    zt[:, 128 * h:128 * (h + 1)],
                    ps_r2[:, 128 * h:128 * (h + 1)], dcoz[:, h:h + 1])
                zq[h].dma_start(z_d.ap()[:, 128 * h:128 * (h + 1)],
                                zt[:, 128 * h:128 * (h + 1)])

    nc.compile()
    return nc


def _host_prep(x, masked_y, W1, b1, Wmu, bmu, Wls, bls, edge_index, use_bf16):
    npdt = _np_dt(use_bf16)
    src = edge_index[0].astype(np.int64)
    dst = edge_index[1].astype(np.int64)

    A = np.zeros((N, N), np.float32)
    np.add.at(A, (src, dst), 1.0)
    idx = np.arange(N)
    A[idx, idx] += 1.0

    # integer degree counts (edges + self loop), replicated
    deg_cnt = np.bincount(dst, minlength=N).astype(np.float32) + 1.0
    cdeg_sw = np.ascontiguousarray(
        deg_cnt.reshape(NT, 128).T).astype(np.float32)

    # full masked_y transposed: myt[p, 1024j + s] = masked_y[s, 128j + p]
    M = masked_y[:HALF, :HALF]
    myt_sw = np.ascontiguousarray(
        M.T.reshape(8, 128, HALF).transpose(1, 0, 2).reshape(128, 8 * HALF)
    ).astype(npdt)

    # full x, global src tiles
    xf_sw = np.ascontiguousarray(
        x.reshape(NT, 128, F).transpose(1, 0, 2).reshape(128, NT * F)
    ).astype(npdt)

    W2 = np.concatenate([Wmu, Wls], axis=1).astype(npdt)
    b1r = np.ascontiguousarray(b1.reshape(1, F)).astype(npdt)
    b2r = np.concatenate([bmu, bls]).reshape(1, F).astype(npdt)
    W1c = np.ascontiguousarray(W1).astype(npdt)

    in_maps = []
    for k in range(NCORES):
        cols = np.r_[128 * k:128 * k + 128, HALF + 128 * k:HALF + 128 * k + 128]
        adj_k = A[:, cols]  # [2048, 256]
        adj_sw = np.ascontiguousarray(
            adj_k.reshape(NT, 128, CPC).transpose(1, 0, 2).reshape(128, NT * CPC)
        ).astype(npdt)
        my_k = masked_y[:HALF, F * k:F * (k + 1)]  # [1024, 128]
        my_sw = np.ascontiguousarray(
            my_k.reshape(8, 128, F).transpose(1, 0, 2).reshape(128, 8 * F)
        ).astype(npdt)
        in_maps.append({
            "adj": adj_sw,
            "my": my_sw,
            "myt": myt_sw,
            "xf": xf_sw,
            "cdeg": cdeg_sw,
            "w1": W1c,
            "w2": W2,
            "b1": b1r,
            "b2": b2r,
        })
    return in_maps


def _assemble(results):
    zfull = np.empty((N, F), np.float32)
    for k in range(NCORES):
        zk = results[k]["z"]  # [128, 256]: [own-node-in-block p, (h, feat)]
        zfull[128 * k:128 * (k + 1)] = zk[:, 0:128]
        zfull[HALF + 128 * k:HALF + 128 * (k + 1)] = zk[:, 128:256]
    return zfull[:, :F // 2].copy(), zfull[:, F // 2:].copy()


def _make_runner(nc):
    """Cached shard_map runner (mirror of bass2jax.run_bass_via_pjrt's
    multi-core branch, minus donation so the jitted fn is reusable)."""
    import jax
    from jax.sharding import Mesh, PartitionSpec
    from jax.experimental.shard_map import shard_map
    from concourse import bass2jax, mybir

    bass2jax.install_neuronx_cc_hook()

    def run(in_maps):
        from concourse import bass2jax as b2j
        results = b2j.run_bass_via_pjrt(nc, in_maps, n_cores=NCORES)
        return results

    return run


def kernel(x, masked_y, W1, b1, Wmu, bmu, Wls, bls, edge_index,
           _trace=False, _warm=True):
    use_bf16 = USE_BF16
    if "nc" not in _COMPILED or _COMPILED.get("bf16") != use_bf16:
        _COMPILED["nc"] = _build_program(use_bf16)
        _COMPILED["bf16"] = use_bf16
        _COMPILED["run"] = _make_runner(_COMPILED["nc"])

    in_maps = _host_prep(
        np.asarray(x, np.float32), np.asarray(masked_y, np.float32),
        np.asarray(W1, np.float32), np.asarray(b1, np.float32),
        np.asarray(Wmu, np.float32), np.asarray(bmu, np.float32),
        np.asarray(Wls, np.float32), np.asarray(bls, np.float32),
        np.asarray(edge_index), use_bf16,
    )
    run = _COMPILED["run"]
    if _warm and not _COMPILED.get("warmed"):
        run(in_maps)  # first call pays NEFF load on every core
        _COMPILED["warmed"] = True
    if _trace:
        import tempfile
        try:
            from antenv import axon_hooks
            hook = axon_hooks.get_axon_ntff_profile_hook()
        except ImportError:
            hook = None
        if hook is None:
            results = run(in_maps)
        else:
            neff_dir = tempfile.mkdtemp()
            with hook(neff_dir, list(range(NCORES))):
                results = run(in_maps)
            _COMPILED["ntff_dir"] = neff_dir
            try:
                import gauge.profiler
                from concourse._compat import FishPath
                from concourse.bass_utils import _process_ntff_profile
                profile = gauge.profiler.Profile(
                    profile_path=FishPath(neff_dir), kernel_dev_mode=True,
                    profile_on_exit=False, bass_kernel=_COMPILED["nc"].m,
                    offline_processing=True, fname="*_body*",
                )
                r = _process_ntff_profile(
                    profile, neff_dir, _COMPILED["nc"], list(range(NCORES)),
                    list(range(NCORES)), False, {}, trace_events=False,
                )
                _COMPILED["exec_time_ns"] = r.exec_time_ns
                _COMPILED["mean_exec_time_ns"] = r.mean_exec_time_ns
            except Exception as e:
                _COMPILED["exec_time_ns"] = None
                _COMPILED["trace_err"] = repr(e)
    else:
        results = run(in_maps)
    return _assemble(results)


# revision 29
# speedup vs baseline: 1.0196x; 1.0196x over previous
"""GCN encoder (2-layer GCN with shared graph) on 8 Trainium2 NeuronCores.

Math (per gcn_conv, PyG GCNConv with edge weights, self-loops in edge list):
    deg[v]  = sum of w over edges (s -> v)            (in-degree, weighted)
    dinv    = deg ** -0.5                             (deg >= 1 always: self-loops)
    out[d]  = dinv[d] * sum_s Wgt[s,d] * dinv[s] * h[s] @ W + b
where Wgt[s,d] = count(edge_index) + I (self loops) + sigmoid(masked_y[:1024,:1024])
          (the sigmoid part only on the [0:1024) x [0:1024) block)

Design: single-collective.  Measurement shows the FIRST collective of a NEFF
execution completes at ~(last core start + 68us) regardless of payload or
trigger time (host-mediated CC-channel bring-up), while later collectives are
fast.  So the kernel uses exactly ONE collective (the unavoidable hidden-state
exchange) and puts everything else before that wall:

  - each core holds: its 256-dst-column adjacency shard (host-counted ints),
    the FULL masked_y^T (2MB bf16, replicated), the FULL x, and the host's
    integer degree counts (replicated).
  - full degree vector is computed redundantly per core: sigmoid(masked_y^T)
    on ACT + free-dim reduce on DVE gives deg_sig in per-partition layout
    [128,8]; + count degrees -> dinv for ALL 2048 nodes, no exchange.
  - dinv[src] is folded into the adjacency rows (adj~ = dinv_s * Wgt); layer 1
    runs fully locally; ONE AllGather exchanges hidden [2048,128] bf16;
    layer 2 runs locally.
  - second-stage matmuls use agg chunks as stationary and W as moving, which
    yields [dst, feat] outputs directly - no PE transposes anywhere.
    out = dinv_d*(agg@W + sqrt(deg_d)*b) with relu(dinv*x)=dinv*relu(x).

All float math (sigmoid, degrees, normalization, aggregation, dense layers)
runs on device; the host only counts integer edges and lays out memory.
"""

import numpy as np

N = 2048
HALF = 1024
F = 128          # IN_C == HID == 128
NCORES = 8
NT = 16          # 16 src-row tiles of 128
CPC = 256        # columns (dst nodes) per core

USE_BF16 = True

_COMPILED = {}


def _np_dt(use_bf16):
    if use_bf16:
        import ml_dtypes
        return np.dtype(ml_dtypes.bfloat16)
    return np.dtype(np.float32)


def _build_program(use_bf16):
    import concourse.bacc as bacc
    import concourse.tile as tile
    from concourse import mybir

    f32 = mybir.dt.float32
    DT = mybir.dt.bfloat16 if use_bf16 else f32
    FP8 = mybir.dt.float8e4
    npdt = _np_dt(use_bf16)
    AF = mybir.ActivationFunctionType
    MUL = mybir.AluOpType.mult
    ADD = mybir.AluOpType.add
    AX = mybir.AxisListType
    # layer 2 runs in fp8 (hidden is post-relu positive; aggregation sums
    # ~1e3 positive terms so fp8 rounding averages out).  Scales keep the
    # small adjacency/hidden values out of the fp8 subnormal range; the
    # combined factor divides back out of the final per-partition scale.
    S_ADJ, S_H = 32.0, 32.0
    S_L2 = S_ADJ * S_H

    nc = bacc.Bacc(
        "TRN2",
        target_bir_lowering=False,
        debug=False,
        enable_asserts=False,
        num_devices=NCORES,
    )

    # I/O (layouts pre-swizzled on host to [128, ...])
    adj_d = nc.dram_tensor("adj", [128, NT * CPC], DT, kind="ExternalInput")
    my_d = nc.dram_tensor("my", [128, 8 * F], DT, kind="ExternalInput")
    myt_d = nc.dram_tensor("myt", [128, 8 * HALF], FP8, kind="ExternalInput")
    xf_d = nc.dram_tensor("xf", [128, NT * F], DT, kind="ExternalInput")
    cdeg_d = nc.dram_tensor("cdeg", [128, NT], f32, kind="ExternalInput")
    w1_d = nc.dram_tensor("w1", [F, F], DT, kind="ExternalInput")
    w2_d = nc.dram_tensor("w2", [F, F], DT, kind="ExternalInput")
    b1_d = nc.dram_tensor("b1", [1, F], DT, kind="ExternalInput")
    b2_d = nc.dram_tensor("b2", [1, F], DT, kind="ExternalInput")
    z_d = nc.dram_tensor("z", [128, 2 * F], f32, kind="ExternalOutput")

    ones_col_d = nc.inline_tensor(np.ones((128, 1), npdt), "ones_col")
    ones11_d = nc.inline_tensor(np.ones((1, 1), np.float32), "ones11")

    rg = [list(range(NCORES))]

    with tile.TileContext(nc) as tc:
        with (
            tc.tile_pool(name="big", bufs=1) as big,
            tc.tile_pool(name="work", bufs=2) as work,
            tc.tile_pool(name="ps", bufs=1, space="PSUM") as ps,
            tc.tile_pool(name="dram", bufs=1, space="DRAM") as dram,
        ):
            # ---- loads ----
            # scalar issues nothing: DMA issues serialize with ACT compute
            # on the same engine and would stall the sigmoid chain.
            onec = big.tile([128, 1], DT, name="onec_sb")
            nc.gpsimd.dma_start(onec[:], ones_col_d.ap())
            ones11 = big.tile([1, 1], f32, name="ones11_sb")
            nc.gpsimd.dma_start(ones11[:], ones11_d.ap())
            cdeg = big.tile([128, NT], f32, name="cdeg_sb")
            nc.gpsimd.dma_start(cdeg[:], cdeg_d.ap())
            myt = big.tile([128, 8 * HALF], FP8, name="myt_sb")
            for j in range(4):
                nc.sync.dma_start(myt[:, 2 * HALF * j:2 * HALF * (j + 1)],
                                  myt_d.ap()[:, 2 * HALF * j:2 * HALF * (j + 1)])
            myb = big.tile([128, 8 * F], DT, name="my_sb")
            nc.gpsimd.dma_start(myb[:], my_d.ap())
            adj = big.tile([128, NT * CPC], DT, name="adj_sb")
            # tiles 8-15 first (degree matmuls need them before the
            # sigmoid-gated tiles 0-7)
            nc.gpsimd.dma_start(adj[:, 8 * CPC:], adj_d.ap()[:, 8 * CPC:])
            nc.gpsimd.dma_start(adj[:, :8 * CPC], adj_d.ap()[:, :8 * CPC])
            xf = big.tile([128, NT * F], DT, name="xf_sb")
            nc.gpsimd.dma_start(xf[:], xf_d.ap())
            w1s = big.tile([F, F], DT, name="w1_sb")
            nc.gpsimd.dma_start(w1s[:], w1_d.ap())
            w2s = big.tile([F, F], DT, name="w2_sb")
            nc.gpsimd.dma_start(w2s[:], w2_d.ap())
            b1s = big.tile([1, F], DT, name="b1_sb")
            nc.gpsimd.dma_start(b1s[:], b1_d.ap())
            b2s = big.tile([1, F], DT, name="b2_sb")
            nc.gpsimd.dma_start(b2s[:], b2_d.ap())

            # ---- full deg_sig: sigmoid(masked_y^T) + free-dim reduce ----
            # degsig[p, j] = sum_s sigmoid(masked_y[s, 128j+p])
            degsig = big.tile([128, 8], f32, name="degsig_sb")
            for j in range(4):
                sgt = work.tile([128, 2 * HALF], DT, tag="sgt")
                nc.scalar.activation(sgt[:], myt[:, 2 * HALF * j:2 * HALF * (j + 1)],
                                     AF.Sigmoid)
                for u in range(2):
                    nc.vector.tensor_reduce(
                        degsig[:, 2 * j + u:2 * j + u + 1],
                        sgt[:, HALF * u:HALF * (u + 1)], axis=AX.X, op=ADD)

            # ---- Wgt dense block: adj += sigmoid(masked_y own-col shard) ----
            sg = work.tile([128, 8 * F], DT, tag="sg")
            nc.scalar.activation(sg[:], myb[:], AF.Sigmoid)
            for i in range(8):
                nc.vector.tensor_add(
                    adj[:, CPC * i:CPC * i + F], adj[:, CPC * i:CPC * i + F],
                    sg[:, F * i:F * (i + 1)])

            # ---- full dinv in per-partition layout [128, 16] ----
            dfull = big.tile([128, 8], f32, name="dfull_sb")
            nc.vector.tensor_tensor(dfull[:], cdeg[:, 0:8], degsig[:], op=ADD)
            sq_bc = big.tile([128, NT], f32, name="sqbc_sb")
            nc.scalar.activation(sq_bc[:, 0:8], dfull[:], AF.Sqrt)
            nc.scalar.activation(sq_bc[:, 8:16], cdeg[:, 8:16], AF.Sqrt)
            dinv_bc = big.tile([128, NT], f32, name="dinvbc_sb")
            nc.vector.reciprocal(dinv_bc[:], sq_bc[:])

            # ---- own-column degree row (for bias trick + dco) ----
            # colsums of the full Wgt shard; tiles 8-15 first (no sigmoid dep)
            ps_deg = ps.tile([1, CPC], f32, name="ps_deg")
            t_order = list(range(8, NT)) + list(range(8))
            for i, t in enumerate(t_order):
                nc.tensor.matmul(
                    ps_deg[:], onec[:], adj[:, CPC * t:CPC * (t + 1)],
                    start=(i == 0), stop=(i == NT - 1),
                )
            sqd = big.tile([1, CPC], f32, name="sqd_sb")     # sqrt(deg) (own)
            nc.scalar.activation(sqd[:], ps_deg[:], AF.Sqrt)
            sqb = big.tile([1, CPC], DT, name="sqb_sb")      # bf16 for bias mm
            nc.vector.tensor_copy(sqb[:], sqd[:])
            # dco[p, h] = dinv[own node 128h+p] via tiny transpose matmuls
            ps_dc = ps.tile([128, 2], f32, name="ps_dc")
            for h in range(2):
                nc.tensor.matmul(ps_dc[:, h:h + 1],
                                 sqd[:, 128 * h:128 * (h + 1)],
                                 ones11[:], start=(h == 0), stop=(h == 1))
            dco = big.tile([128, 2], f32, name="dco_sb")
            nc.vector.reciprocal(dco[:], ps_dc[:])

            # ---- fold dinv[src] into adjacency rows: adj~ = dinv_s * Wgt ----
            for t in range(NT):
                nc.vector.tensor_scalar_mul(
                    adj[:, CPC * t:CPC * (t + 1)], adj[:, CPC * t:CPC * (t + 1)],
                    dinv_bc[:, t:t + 1],
                )
            # fp8 copy of adj~ for layer 2
            adj8 = big.tile([128, NT * CPC], FP8, name="adj8_sb")
            nc.vector.tensor_scalar_mul(adj8[:], adj[:], S_ADJ)
            dco8 = big.tile([128, 2], f32, name="dco8_sb")
            nc.vector.tensor_scalar_mul(dco8[:], dco[:], S_H)
            dcoz = big.tile([128, 2], f32, name="dcoz_sb")
            nc.vector.tensor_scalar_mul(dcoz[:], dco[:], 1.0 / S_L2)
            sqbl2 = big.tile([1, CPC], DT, name="sqbl2_sb")
            nc.vector.tensor_scalar_mul(sqbl2[:], sqd[:], S_L2)

            def layer(xtiles, adjt, wsb, bsb, sqrow, name):
                # aggT[f, d] = sum_s x[s, f] * adj~[s, d]
                ps_agg = ps.tile([128, CPC], f32, name=f"ps_agg_{name}",
                                 tag="ps_agg")
                for t in range(NT):
                    nc.tensor.matmul(
                        ps_agg[:], xtiles[:, F * t:F * (t + 1)],
                        adjt[:, CPC * t:CPC * (t + 1)],
                        start=(t == 0), stop=(t == NT - 1),
                    )
                # r[d, o] = sum_f aggT[f, d]*W[f, o] + sqrt(deg_d)*b[o]
                # (agg chunk as stationary -> [dst, feat] directly); the
                # PSUM->SBUF cast is split per half so the first half's W
                # matmul overlaps the second half's cast.
                aggs = work.tile([128, CPC], DT, tag="aggs")
                ps_r = ps.tile([128, CPC], f32, name=f"ps_r_{name}", tag="ps_r")
                for h in range(2):
                    nc.vector.tensor_copy(aggs[:, 128 * h:128 * (h + 1)],
                                          ps_agg[:, 128 * h:128 * (h + 1)])
                    nc.tensor.matmul(ps_r[:, 128 * h:128 * (h + 1)],
                                     aggs[:, 128 * h:128 * (h + 1)], wsb[:],
                                     start=True, stop=False)
                    nc.tensor.matmul(ps_r[:, 128 * h:128 * (h + 1)],
                                     sqrow[:, 128 * h:128 * (h + 1)], bsb[:],
                                     start=False, stop=True)
                return ps_r

            # ---- layer 1 (fully local) ----
            ps_r1 = layer(xf, adj, w1s, b1s, sqb, "l1")
            # hidden[d,o] = relu(dinv_d*r) = dinv_d*relu(r)   (dinv > 0)
            g01 = work.tile([128, 2 * F], FP8, tag="g01")
            rl1 = work.tile([128, 2 * F], DT, tag="rl1")
            for h in range(2):
                nc.scalar.activation(rl1[:, 128 * h:128 * (h + 1)],
                                     ps_r1[:, 128 * h:128 * (h + 1)], AF.Relu)
                nc.vector.tensor_scalar_mul(
                    g01[:, 128 * h:128 * (h + 1)],
                    rl1[:, 128 * h:128 * (h + 1)], dco8[:, h:h + 1])

            # ---- the single collective: gather hidden [2048, 128] fp8 ----
            ag_in = dram.tile([CPC, F], FP8, name="ag_in")
            ag_out = dram.tile([N, F], FP8, name="ag_out", addr_space="Shared")
            nc.scalar.dma_start(ag_in[:].rearrange("(h p) c -> p h c", h=2), g01[:])
            nc.gpsimd.collective_compute(
                "AllGather", mybir.AluOpType.bypass,
                replica_groups=rg, ins=[ag_in.opt()], outs=[ag_out.opt()],
            )

            # gathered row 256k+128h+p = hidden[node 1024h+128k+p]; src tile
            # t=8h+k is a contiguous [128,128] block -> 3 parallel reloads
            xb2 = big.tile([128, NT * F], FP8, name="xb2_sb")
            v = ag_out[:].rearrange("(k h p) c -> h p k c", k=8, h=2)
            # one issue per queue, smallest first so matmul t=0 starts
            # while the rest still streams
            nc.scalar.dma_start(xb2[:, 0:2 * F], v[0][:, 0:2])
            nc.sync.dma_start(xb2[:, 2 * F:8 * F], v[0][:, 2:8])
            nc.gpsimd.dma_start(xb2[:, 8 * F:16 * F], v[1])

            # ---- layer 2 ----
            ps_r2 = layer(xb2, adj8, w2s, b2s, sqbl2, "l2")
            zt = work.tile([128, 2 * F], f32, tag="zt")
            zq = [nc.scalar, nc.sync]
            for h in range(2):
                nc.vector.tensor_scalar_mul(
                    zt[:, 128 * h:128 * (h + 1)],
                    ps_r2[:, 128 * h:128 * (h + 1)], dcoz[:, h:h + 1])
                zq[h].dma_start(z_d.ap()[:, 128 * h:128 * (h + 1)],
                                zt[:, 128 * h:128 * (h + 1)])

    nc.compile()
    return nc


def _host_prep(x, masked_y, W1, b1, Wmu, bmu, Wls, bls, edge_index, use_bf16):
    npdt = _np_dt(use_bf16)
    src = edge_index[0].astype(np.int64)
    dst = edge_index[1].astype(np.int64)

    A = np.zeros((N, N), np.float32)
    np.add.at(A, (src, dst), 1.0)
    idx = np.arange(N)
    A[idx, idx] += 1.0

    # integer degree counts (edges + self loop), replicated
    deg_cnt = np.bincount(dst, minlength=N).astype(np.float32) + 1.0
    cdeg_sw = np.ascontiguousarray(
        deg_cnt.reshape(NT, 128).T).astype(np.float32)

    # full masked_y transposed: myt[p, 1024j + s] = masked_y[s, 128j + p]
    # fp8 is plenty for the degree path: sigmoid errors average out over the
    # 1024-term column sums (the adjacency dense block stays bf16)
    import ml_dtypes
    M = masked_y[:HALF, :HALF]
    myt_sw = np.ascontiguousarray(
        M.T.reshape(8, 128, HALF).transpose(1, 0, 2).reshape(128, 8 * HALF)
    ).astype(ml_dtypes.float8_e4m3)

    # full x, global src tiles
    xf_sw = np.ascontiguousarray(
        x.reshape(NT, 128, F).transpose(1, 0, 2).reshape(128, NT * F)
    ).astype(npdt)

    W2 = np.concatenate([Wmu, Wls], axis=1).astype(npdt)
    b1r = np.ascontiguousarray(b1.reshape(1, F)).astype(npdt)
    b2r = np.concatenate([bmu, bls]).reshape(1, F).astype(npdt)
    W1c = np.ascontiguousarray(W1).astype(npdt)

    in_maps = []
    for k in range(NCORES):
        cols = np.r_[128 * k:128 * k + 128, HALF + 128 * k:HALF + 128 * k + 128]
        adj_k = A[:, cols]  # [2048, 256]
        adj_sw = np.ascontiguousarray(
            adj_k.reshape(NT, 128, CPC).transpose(1, 0, 2).reshape(128, NT * CPC)
        ).astype(npdt)
        my_k = masked_y[:HALF, F * k:F * (k + 1)]  # [1024, 128]
        my_sw = np.ascontiguousarray(
            my_k.reshape(8, 128, F).transpose(1, 0, 2).reshape(128, 8 * F)
        ).astype(npdt)
        in_maps.append({
            "adj": adj_sw,
            "my": my_sw,
            "myt": myt_sw,
            "xf": xf_sw,
            "cdeg": cdeg_sw,
            "w1": W1c,
            "w2": W2,
            "b1": b1r,
            "b2": b2r,
        })
    return in_maps


def _assemble(results):
    zfull = np.empty((N, F), np.float32)
    for k in range(NCORES):
        zk = results[k]["z"]  # [128, 256]: [own-node-in-block p, (h, feat)]
        zfull[128 * k:128 * (k + 1)] = zk[:, 0:128]
        zfull[HALF + 128 * k:HALF + 128 * (k + 1)] = zk[:, 128:256]
    return zfull[:, :F // 2].copy(), zfull[:, F // 2:].copy()


def _make_runner(nc):
    """Cached shard_map runner (mirror of bass2jax.run_bass_via_pjrt's
    multi-core branch, minus donation so the jitted fn is reusable)."""
    import jax
    from jax.sharding import Mesh, PartitionSpec
    from jax.experimental.shard_map import shard_map
    from concourse import bass2jax, mybir

    bass2jax.install_neuronx_cc_hook()

    def run(in_maps):
        from concourse import bass2jax as b2j
        results = b2j.run_bass_via_pjrt(nc, in_maps, n_cores=NCORES)
        return results

    return run


def kernel(x, masked_y, W1, b1, Wmu, bmu, Wls, bls, edge_index,
           _trace=False, _warm=True):
    use_bf16 = USE_BF16
    if "nc" not in _COMPILED or _COMPILED.get("bf16") != use_bf16:
        _COMPILED["nc"] = _build_program(use_bf16)
        _COMPILED["bf16"] = use_bf16
        _COMPILED["run"] = _make_runner(_COMPILED["nc"])

    in_maps = _host_prep(
        np.asarray(x, np.float32), np.asarray(masked_y, np.float32),
        np.asarray(W1, np.float32), np.asarray(b1, np.float32),
        np.asarray(Wmu, np.float32), np.asarray(bmu, np.float32),
        np.asarray(Wls, np.float32), np.asarray(bls, np.float32),
        np.asarray(edge_index), use_bf16,
    )
    run = _COMPILED["run"]
    if _warm and not _COMPILED.get("warmed"):
        run(in_maps)  # first call pays NEFF load on every core
        _COMPILED["warmed"] = True
    if _trace:
        import tempfile
        try:
            from antenv import axon_hooks
            hook = axon_hooks.get_axon_ntff_profile_hook()
        except ImportError:
            hook = None
        if hook is None:
            results = run(in_maps)
        else:
            neff_dir = tempfile.mkdtemp()
            with hook(neff_dir, list(range(NCORES))):
                results = run(in_maps)
            _COMPILED["ntff_dir"] = neff_dir
            try:
                import gauge.profiler
                from concourse._compat import FishPath
                from concourse.bass_utils import _process_ntff_profile
                profile = gauge.profiler.Profile(
                    profile_path=FishPath(neff_dir), kernel_dev_mode=True,
                    profile_on_exit=False, bass_kernel=_COMPILED["nc"].m,
                    offline_processing=True, fname="*_body*",
                )
                r = _process_ntff_profile(
                    profile, neff_dir, _COMPILED["nc"], list(range(NCORES)),
                    list(range(NCORES)), False, {}, trace_events=False,
                )
                _COMPILED["exec_time_ns"] = r.exec_time_ns
                _COMPILED["mean_exec_time_ns"] = r.mean_exec_time_ns
            except Exception as e:
                _COMPILED["exec_time_ns"] = None
                _COMPILED["trace_err"] = repr(e)
    else:
        results = run(in_maps)
    return _assemble(results)


# revision 31
# speedup vs baseline: 1.0772x; 1.0565x over previous
"""GCN encoder (2-layer GCN with shared graph) on 8 Trainium2 NeuronCores.

Math (per gcn_conv, PyG GCNConv with edge weights, self-loops in edge list):
    deg[v]  = sum of w over edges (s -> v)            (in-degree, weighted)
    dinv    = deg ** -0.5                             (deg >= 1 always: self-loops)
    out[d]  = dinv[d] * sum_s Wgt[s,d] * dinv[s] * h[s] @ W + b
where Wgt[s,d] = count(edge_index) + I (self loops) + sigmoid(masked_y[:1024,:1024])
          (the sigmoid part only on the [0:1024) x [0:1024) block)

Design: single-collective.  Measurement shows the FIRST collective of a NEFF
execution completes at ~(last core start + 68us) regardless of payload or
trigger time (host-mediated CC-channel bring-up), while later collectives are
fast.  So the kernel uses exactly ONE collective (the unavoidable hidden-state
exchange) and puts everything else before that wall:

  - each core holds: its 256-dst-column adjacency shard (host-counted ints),
    the FULL masked_y^T (2MB bf16, replicated), the FULL x, and the host's
    integer degree counts (replicated).
  - full degree vector is computed redundantly per core: sigmoid(masked_y^T)
    on ACT + free-dim reduce on DVE gives deg_sig in per-partition layout
    [128,8]; + count degrees -> dinv for ALL 2048 nodes, no exchange.
  - dinv[src] is folded into the adjacency rows (adj~ = dinv_s * Wgt); layer 1
    runs fully locally; ONE AllGather exchanges hidden [2048,128] bf16;
    layer 2 runs locally.
  - second-stage matmuls use agg chunks as stationary and W as moving, which
    yields [dst, feat] outputs directly - no PE transposes anywhere.
    out = dinv_d*(agg@W + sqrt(deg_d)*b) with relu(dinv*x)=dinv*relu(x).

All float math (sigmoid, degrees, normalization, aggregation, dense layers)
runs on device; the host only counts integer edges and lays out memory.
"""

import numpy as np

N = 2048
HALF = 1024
F = 128          # IN_C == HID == 128
NCORES = 8
NT = 16          # 16 src-row tiles of 128
CPC = 256        # columns (dst nodes) per core

USE_BF16 = True
USE_FP8_PAYLOAD = False

_COMPILED = {}


def _np_dt(use_bf16):
    if use_bf16:
        import ml_dtypes
        return np.dtype(ml_dtypes.bfloat16)
    return np.dtype(np.float32)


def _build_program(use_bf16):
    import concourse.bacc as bacc
    import concourse.tile as tile
    from concourse import mybir

    f32 = mybir.dt.float32
    DT = mybir.dt.bfloat16 if use_bf16 else f32
    FP8 = mybir.dt.float8e4
    PAY = FP8 if USE_FP8_PAYLOAD else DT
    npdt = _np_dt(use_bf16)
    AF = mybir.ActivationFunctionType
    MUL = mybir.AluOpType.mult
    ADD = mybir.AluOpType.add
    AX = mybir.AxisListType
    # layer 2 runs in fp8 (hidden is post-relu positive; aggregation sums
    # ~1e3 positive terms so fp8 rounding averages out).  Scales keep the
    # small adjacency/hidden values out of the fp8 subnormal range; the
    # combined factor divides back out of the final per-partition scale.
    S_ADJ, S_H = 32.0, 32.0
    S_L2 = S_ADJ * S_H

    nc = bacc.Bacc(
        "TRN2",
        target_bir_lowering=False,
        debug=False,
        enable_asserts=False,
        num_devices=NCORES,
    )

    # I/O (layouts pre-swizzled on host to [128, ...])
    adj_d = nc.dram_tensor("adj", [128, NT * CPC], DT, kind="ExternalInput")
    my_d = nc.dram_tensor("my", [128, 8 * F], DT, kind="ExternalInput")
    myt_d = nc.dram_tensor("myt", [128, 8 * HALF], FP8, kind="ExternalInput")
    xf_d = nc.dram_tensor("xf", [128, NT * F], DT, kind="ExternalInput")
    cdeg_d = nc.dram_tensor("cdeg", [128, NT], f32, kind="ExternalInput")
    w1_d = nc.dram_tensor("w1", [F, F], DT, kind="ExternalInput")
    w2_d = nc.dram_tensor("w2", [F, F], DT, kind="ExternalInput")
    b1_d = nc.dram_tensor("b1", [1, F], DT, kind="ExternalInput")
    b2_d = nc.dram_tensor("b2", [1, F], DT, kind="ExternalInput")
    z_d = nc.dram_tensor("z", [128, 2 * F], f32, kind="ExternalOutput")

    ones_col_d = nc.inline_tensor(np.ones((128, 1), npdt), "ones_col")
    ones11_d = nc.inline_tensor(np.ones((1, 1), np.float32), "ones11")

    rg = [list(range(NCORES))]

    with tile.TileContext(nc) as tc:
        with (
            tc.tile_pool(name="big", bufs=1) as big,
            tc.tile_pool(name="work", bufs=2) as work,
            tc.tile_pool(name="ps", bufs=1, space="PSUM") as ps,
            tc.tile_pool(name="dram", bufs=1, space="DRAM") as dram,
        ):
            # ---- loads ----
            # scalar issues nothing: DMA issues serialize with ACT compute
            # on the same engine and would stall the sigmoid chain.
            onec = big.tile([128, 1], DT, name="onec_sb")
            nc.gpsimd.dma_start(onec[:], ones_col_d.ap())
            ones11 = big.tile([1, 1], f32, name="ones11_sb")
            nc.gpsimd.dma_start(ones11[:], ones11_d.ap())
            cdeg = big.tile([128, NT], f32, name="cdeg_sb")
            nc.gpsimd.dma_start(cdeg[:], cdeg_d.ap())
            myt = big.tile([128, 8 * HALF], FP8, name="myt_sb")
            for j in range(4):
                nc.sync.dma_start(myt[:, 2 * HALF * j:2 * HALF * (j + 1)],
                                  myt_d.ap()[:, 2 * HALF * j:2 * HALF * (j + 1)])
            myb = big.tile([128, 8 * F], DT, name="my_sb")
            nc.gpsimd.dma_start(myb[:], my_d.ap())
            adj = big.tile([128, NT * CPC], DT, name="adj_sb")
            # tiles 8-15 first (degree matmuls need them before the
            # sigmoid-gated tiles 0-7)
            nc.gpsimd.dma_start(adj[:, 8 * CPC:], adj_d.ap()[:, 8 * CPC:])
            nc.gpsimd.dma_start(adj[:, :8 * CPC], adj_d.ap()[:, :8 * CPC])
            xf = big.tile([128, NT * F], DT, name="xf_sb")
            nc.gpsimd.dma_start(xf[:], xf_d.ap())
            w1s = big.tile([F, F], DT, name="w1_sb")
            nc.gpsimd.dma_start(w1s[:], w1_d.ap())
            w2s = big.tile([F, F], DT, name="w2_sb")
            nc.gpsimd.dma_start(w2s[:], w2_d.ap())
            b1s = big.tile([1, F], DT, name="b1_sb")
            nc.gpsimd.dma_start(b1s[:], b1_d.ap())
            b2s = big.tile([1, F], DT, name="b2_sb")
            nc.gpsimd.dma_start(b2s[:], b2_d.ap())

            # ---- full deg_sig: sigmoid(masked_y^T) + free-dim reduce ----
            # degsig[p, j] = sum_s sigmoid(masked_y[s, 128j+p])
            degsig = big.tile([128, 8], f32, name="degsig_sb")
            for j in range(4):
                sgt = work.tile([128, 2 * HALF], DT, tag="sgt")
                nc.scalar.activation(sgt[:], myt[:, 2 * HALF * j:2 * HALF * (j + 1)],
                                     AF.Sigmoid)
                for u in range(2):
                    nc.vector.tensor_reduce(
                        degsig[:, 2 * j + u:2 * j + u + 1],
                        sgt[:, HALF * u:HALF * (u + 1)], axis=AX.X, op=ADD)

            # ---- Wgt dense block: adj += sigmoid(masked_y own-col shard) ----
            sg = work.tile([128, 8 * F], DT, tag="sg")
            nc.scalar.activation(sg[:], myb[:], AF.Sigmoid)
            for i in range(8):
                nc.vector.tensor_add(
                    adj[:, CPC * i:CPC * i + F], adj[:, CPC * i:CPC * i + F],
                    sg[:, F * i:F * (i + 1)])

            # ---- full dinv in per-partition layout [128, 16] ----
            dfull = big.tile([128, 8], f32, name="dfull_sb")
            nc.vector.tensor_tensor(dfull[:], cdeg[:, 0:8], degsig[:], op=ADD)
            sq_bc = big.tile([128, NT], f32, name="sqbc_sb")
            nc.scalar.activation(sq_bc[:, 0:8], dfull[:], AF.Sqrt)
            nc.scalar.activation(sq_bc[:, 8:16], cdeg[:, 8:16], AF.Sqrt)
            dinv_bc = big.tile([128, NT], f32, name="dinvbc_sb")
            nc.vector.reciprocal(dinv_bc[:], sq_bc[:])

            # ---- own-column degree row (for bias trick + dco) ----
            # colsums of the full Wgt shard; tiles 8-15 first (no sigmoid dep)
            ps_deg = ps.tile([1, CPC], f32, name="ps_deg")
            t_order = list(range(8, NT)) + list(range(8))
            for i, t in enumerate(t_order):
                nc.tensor.matmul(
                    ps_deg[:], onec[:], adj[:, CPC * t:CPC * (t + 1)],
                    start=(i == 0), stop=(i == NT - 1),
                )
            sqd = big.tile([1, CPC], f32, name="sqd_sb")     # sqrt(deg) (own)
            nc.scalar.activation(sqd[:], ps_deg[:], AF.Sqrt)
            sqb = big.tile([1, CPC], DT, name="sqb_sb")      # bf16 for bias mm
            nc.vector.tensor_copy(sqb[:], sqd[:])
            # dco[p, h] = dinv[own node 128h+p] via tiny transpose matmuls
            ps_dc = ps.tile([128, 2], f32, name="ps_dc")
            for h in range(2):
                nc.tensor.matmul(ps_dc[:, h:h + 1],
                                 sqd[:, 128 * h:128 * (h + 1)],
                                 ones11[:], start=(h == 0), stop=(h == 1))
            dco = big.tile([128, 2], f32, name="dco_sb")
            nc.vector.reciprocal(dco[:], ps_dc[:])

            # ---- fold dinv[src] into adjacency rows: adj~ = dinv_s * Wgt ----
            for t in range(NT):
                nc.vector.tensor_scalar_mul(
                    adj[:, CPC * t:CPC * (t + 1)], adj[:, CPC * t:CPC * (t + 1)],
                    dinv_bc[:, t:t + 1],
                )
            # fp8 copy of adj~ for layer 2
            adj8 = big.tile([128, NT * CPC], PAY, name="adj8_sb")
            nc.vector.tensor_scalar_mul(adj8[:], adj[:], S_ADJ)
            dco8 = big.tile([128, 2], f32, name="dco8_sb")
            nc.vector.tensor_scalar_mul(dco8[:], dco[:], S_H)
            dcoz = big.tile([128, 2], f32, name="dcoz_sb")
            nc.vector.tensor_scalar_mul(dcoz[:], dco[:], 1.0 / S_L2)
            sqbl2 = big.tile([1, CPC], DT, name="sqbl2_sb")
            nc.vector.tensor_scalar_mul(sqbl2[:], sqd[:], S_L2)

            def layer(xtiles, adjt, wsb, bsb, sqrow, name):
                # aggT[f, d] = sum_s x[s, f] * adj~[s, d]
                ps_agg = ps.tile([128, CPC], f32, name=f"ps_agg_{name}",
                                 tag="ps_agg")
                for t in range(NT):
                    nc.tensor.matmul(
                        ps_agg[:], xtiles[:, F * t:F * (t + 1)],
                        adjt[:, CPC * t:CPC * (t + 1)],
                        start=(t == 0), stop=(t == NT - 1),
                    )
                # r[d, o] = sum_f aggT[f, d]*W[f, o] + sqrt(deg_d)*b[o]
                # (agg chunk as stationary -> [dst, feat] directly); the
                # PSUM->SBUF cast is split per half so the first half's W
                # matmul overlaps the second half's cast.
                aggs = work.tile([128, CPC], DT, tag="aggs")
                ps_r = ps.tile([128, CPC], f32, name=f"ps_r_{name}", tag="ps_r")
                for h in range(2):
                    nc.vector.tensor_copy(aggs[:, 128 * h:128 * (h + 1)],
                                          ps_agg[:, 128 * h:128 * (h + 1)])
                    nc.tensor.matmul(ps_r[:, 128 * h:128 * (h + 1)],
                                     aggs[:, 128 * h:128 * (h + 1)], wsb[:],
                                     start=True, stop=False)
                    nc.tensor.matmul(ps_r[:, 128 * h:128 * (h + 1)],
                                     sqrow[:, 128 * h:128 * (h + 1)], bsb[:],
                                     start=False, stop=True)
                return ps_r

            # ---- layer 1 (fully local) ----
            ps_r1 = layer(xf, adj, w1s, b1s, sqb, "l1")
            # hidden[d,o] = relu(dinv_d*r) = dinv_d*relu(r)   (dinv > 0)
            g01 = work.tile([128, 2 * F], PAY, tag="g01")
            rl1 = work.tile([128, 2 * F], DT, tag="rl1")
            for h in range(2):
                nc.scalar.activation(rl1[:, 128 * h:128 * (h + 1)],
                                     ps_r1[:, 128 * h:128 * (h + 1)], AF.Relu)
                nc.vector.tensor_scalar_mul(
                    g01[:, 128 * h:128 * (h + 1)],
                    rl1[:, 128 * h:128 * (h + 1)], dco8[:, h:h + 1])

            # ---- the single collective: gather hidden [2048, 128] fp8 ----
            ag_in = dram.tile([CPC, F], PAY, name="ag_in")
            ag_out = dram.tile([N, F], PAY, name="ag_out", addr_space="Shared")
            nc.scalar.dma_start(ag_in[:].rearrange("(h p) c -> p h c", h=2), g01[:])
            nc.gpsimd.collective_compute(
                "AllGather", mybir.AluOpType.bypass,
                replica_groups=rg, ins=[ag_in.opt()], outs=[ag_out.opt()],
            )

            # gathered row 256k+128h+p = hidden[node 1024h+128k+p]; src tile
            # t=8h+k is a contiguous [128,128] block -> 3 parallel reloads
            xb2 = big.tile([128, NT * F], PAY, name="xb2_sb")
            v = ag_out[:].rearrange("(k h p) c -> h p k c", k=8, h=2)
            # one issue per queue, smallest first so matmul t=0 starts
            # while the rest still streams
            nc.scalar.dma_start(xb2[:, 0:2 * F], v[0][:, 0:2])
            nc.sync.dma_start(xb2[:, 2 * F:8 * F], v[0][:, 2:8])
            nc.gpsimd.dma_start(xb2[:, 8 * F:16 * F], v[1])

            # ---- layer 2 ----
            ps_r2 = layer(xb2, adj8, w2s, b2s, sqbl2, "l2")
            zt = work.tile([128, 2 * F], f32, tag="zt")
            zq = [nc.scalar, nc.sync]
            for h in range(2):
                nc.vector.tensor_scalar_mul(
                    zt[:, 128 * h:128 * (h + 1)],
                    ps_r2[:, 128 * h:128 * (h + 1)], dcoz[:, h:h + 1])
                zq[h].dma_start(z_d.ap()[:, 128 * h:128 * (h + 1)],
                                zt[:, 128 * h:128 * (h + 1)])

    nc.compile()
    return nc


def _host_prep(x, masked_y, W1, b1, Wmu, bmu, Wls, bls, edge_index, use_bf16):
    npdt = _np_dt(use_bf16)
    src = edge_index[0].astype(np.int64)
    dst = edge_index[1].astype(np.int64)

    A = np.zeros((N, N), np.float32)
    np.add.at(A, (src, dst), 1.0)
    idx = np.arange(N)
    A[idx, idx] += 1.0

    # integer degree counts (edges + self loop), replicated
    deg_cnt = np.bincount(dst, minlength=N).astype(np.float32) + 1.0
    cdeg_sw = np.ascontiguousarray(
        deg_cnt.reshape(NT, 128).T).astype(np.float32)

    # full masked_y transposed: myt[p, 1024j + s] = masked_y[s, 128j + p]
    # fp8 is plenty for the degree path: sigmoid errors average out over the
    # 1024-term column sums (the adjacency dense block stays bf16)
    import ml_dtypes
    M = masked_y[:HALF, :HALF]
    myt_sw = np.ascontiguousarray(
        M.T.reshape(8, 128, HALF).transpose(1, 0, 2).reshape(128, 8 * HALF)
    ).astype(ml_dtypes.float8_e4m3)

    # full x, global src tiles
    xf_sw = np.ascontiguousarray(
        x.reshape(NT, 128, F).transpose(1, 0, 2).reshape(128, NT * F)
    ).astype(npdt)

    W2 = np.concatenate([Wmu, Wls], axis=1).astype(npdt)
    b1r = np.ascontiguousarray(b1.reshape(1, F)).astype(npdt)
    b2r = np.concatenate([bmu, bls]).reshape(1, F).astype(npdt)
    W1c = np.ascontiguousarray(W1).astype(npdt)

    in_maps = []
    for k in range(NCORES):
        cols = np.r_[128 * k:128 * k + 128, HALF + 128 * k:HALF + 128 * k + 128]
        adj_k = A[:, cols]  # [2048, 256]
        adj_sw = np.ascontiguousarray(
            adj_k.reshape(NT, 128, CPC).transpose(1, 0, 2).reshape(128, NT * CPC)
        ).astype(npdt)
        my_k = masked_y[:HALF, F * k:F * (k + 1)]  # [1024, 128]
        my_sw = np.ascontiguousarray(
            my_k.reshape(8, 128, F).transpose(1, 0, 2).reshape(128, 8 * F)
        ).astype(npdt)
        in_maps.append({
            "adj": adj_sw,
            "my": my_sw,
            "myt": myt_sw,
            "xf": xf_sw,
            "cdeg": cdeg_sw,
            "w1": W1c,
            "w2": W2,
            "b1": b1r,
            "b2": b2r,
        })
    return in_maps


def _assemble(results):
    zfull = np.empty((N, F), np.float32)
    for k in range(NCORES):
        zk = results[k]["z"]  # [128, 256]: [own-node-in-block p, (h, feat)]
        zfull[128 * k:128 * (k + 1)] = zk[:, 0:128]
        zfull[HALF + 128 * k:HALF + 128 * (k + 1)] = zk[:, 128:256]
    return zfull[:, :F // 2].copy(), zfull[:, F // 2:].copy()


def _make_runner(nc):
    """Cached shard_map runner (mirror of bass2jax.run_bass_via_pjrt's
    multi-core branch, minus donation so the jitted fn is reusable)."""
    import jax
    from jax.sharding import Mesh, PartitionSpec
    from jax.experimental.shard_map import shard_map
    from concourse import bass2jax, mybir

    bass2jax.install_neuronx_cc_hook()

    def run(in_maps):
        from concourse import bass2jax as b2j
        results = b2j.run_bass_via_pjrt(nc, in_maps, n_cores=NCORES)
        return results

    return run


def kernel(x, masked_y, W1, b1, Wmu, bmu, Wls, bls, edge_index,
           _trace=False, _warm=True):
    use_bf16 = USE_BF16
    if "nc" not in _COMPILED or _COMPILED.get("bf16") != use_bf16:
        _COMPILED["nc"] = _build_program(use_bf16)
        _COMPILED["bf16"] = use_bf16
        _COMPILED["run"] = _make_runner(_COMPILED["nc"])

    in_maps = _host_prep(
        np.asarray(x, np.float32), np.asarray(masked_y, np.float32),
        np.asarray(W1, np.float32), np.asarray(b1, np.float32),
        np.asarray(Wmu, np.float32), np.asarray(bmu, np.float32),
        np.asarray(Wls, np.float32), np.asarray(bls, np.float32),
        np.asarray(edge_index), use_bf16,
    )
    run = _COMPILED["run"]
    if _warm and not _COMPILED.get("warmed"):
        run(in_maps)  # first call pays NEFF load on every core
        _COMPILED["warmed"] = True
    if _trace:
        import tempfile
        try:
            from antenv import axon_hooks
            hook = axon_hooks.get_axon_ntff_profile_hook()
        except ImportError:
            hook = None
        if hook is None:
            results = run(in_maps)
        else:
            neff_dir = tempfile.mkdtemp()
            with hook(neff_dir, list(range(NCORES))):
                results = run(in_maps)
            _COMPILED["ntff_dir"] = neff_dir
            try:
                import gauge.profiler
                from concourse._compat import FishPath
                from concourse.bass_utils import _process_ntff_profile
                profile = gauge.profiler.Profile(
                    profile_path=FishPath(neff_dir), kernel_dev_mode=True,
                    profile_on_exit=False, bass_kernel=_COMPILED["nc"].m,
                    offline_processing=True, fname="*_body*",
                )
                r = _process_ntff_profile(
                    profile, neff_dir, _COMPILED["nc"], list(range(NCORES)),
                    list(range(NCORES)), False, {}, trace_events=False,
                )
                _COMPILED["exec_time_ns"] = r.exec_time_ns
                _COMPILED["mean_exec_time_ns"] = r.mean_exec_time_ns
            except Exception as e:
                _COMPILED["exec_time_ns"] = None
                _COMPILED["trace_err"] = repr(e)
    else:
        results = run(in_maps)
    return _assemble(results)


# revision 32
# speedup vs baseline: 1.1778x; 1.0934x over previous
"""GCN encoder (2-layer GCN with shared graph) on 8 Trainium2 NeuronCores.

Math (per gcn_conv, PyG GCNConv with edge weights, self-loops in edge list):
    deg[v]  = sum of w over edges (s -> v)            (in-degree, weighted)
    dinv    = deg ** -0.5                             (deg >= 1 always: self-loops)
    out[d]  = dinv[d] * sum_s Wgt[s,d] * dinv[s] * h[s] @ W + b
where Wgt[s,d] = count(edge_index) + I (self loops) + sigmoid(masked_y[:1024,:1024])
          (the sigmoid part only on the [0:1024) x [0:1024) block)

Design: single-collective.  Measurement shows the FIRST collective of a NEFF
execution completes at ~(last core start + 68us) regardless of payload or
trigger time (host-mediated CC-channel bring-up), while later collectives are
fast.  So the kernel uses exactly ONE collective (the unavoidable hidden-state
exchange) and puts everything else before that wall:

  - each core holds: its 256-dst-column adjacency shard (host-counted ints),
    the FULL masked_y^T (2MB bf16, replicated), the FULL x, and the host's
    integer degree counts (replicated).
  - full degree vector is computed redundantly per core: sigmoid(masked_y^T)
    on ACT + free-dim reduce on DVE gives deg_sig in per-partition layout
    [128,8]; + count degrees -> dinv for ALL 2048 nodes, no exchange.
  - dinv[src] is folded into the adjacency rows (adj~ = dinv_s * Wgt); layer 1
    runs fully locally; ONE AllGather exchanges hidden [2048,128] bf16;
    layer 2 runs locally.
  - second-stage matmuls use agg chunks as stationary and W as moving, which
    yields [dst, feat] outputs directly - no PE transposes anywhere.
    out = dinv_d*(agg@W + sqrt(deg_d)*b) with relu(dinv*x)=dinv*relu(x).

All float math (sigmoid, degrees, normalization, aggregation, dense layers)
runs on device; the host only counts integer edges and lays out memory.
"""

import numpy as np

N = 2048
HALF = 1024
F = 128          # IN_C == HID == 128
NCORES = 8
NT = 16          # 16 src-row tiles of 128
CPC = 256        # columns (dst nodes) per core

USE_BF16 = True
USE_FP8_PAYLOAD = True

_COMPILED = {}


def _np_dt(use_bf16):
    if use_bf16:
        import ml_dtypes
        return np.dtype(ml_dtypes.bfloat16)
    return np.dtype(np.float32)


def _build_program(use_bf16):
    import concourse.bacc as bacc
    import concourse.tile as tile
    from concourse import mybir

    f32 = mybir.dt.float32
    DT = mybir.dt.bfloat16 if use_bf16 else f32
    FP8 = mybir.dt.float8e4
    PAY = FP8 if USE_FP8_PAYLOAD else DT
    npdt = _np_dt(use_bf16)
    AF = mybir.ActivationFunctionType
    MUL = mybir.AluOpType.mult
    ADD = mybir.AluOpType.add
    AX = mybir.AxisListType
    # layer 2 runs in fp8 (hidden is post-relu positive; aggregation sums
    # ~1e3 positive terms so fp8 rounding averages out).  Scales keep the
    # small adjacency/hidden values out of the fp8 subnormal range; the
    # combined factor divides back out of the final per-partition scale.
    S_ADJ, S_H = 32.0, 32.0
    S_L2 = S_ADJ * S_H

    nc = bacc.Bacc(
        "TRN2",
        target_bir_lowering=False,
        debug=False,
        enable_asserts=False,
        num_devices=NCORES,
    )

    # I/O (layouts pre-swizzled on host to [128, ...])
    adj_d = nc.dram_tensor("adj", [128, NT * CPC], DT, kind="ExternalInput")
    my_d = nc.dram_tensor("my", [128, 8 * F], DT, kind="ExternalInput")
    myt_d = nc.dram_tensor("myt", [128, 8 * HALF], FP8, kind="ExternalInput")
    xf_d = nc.dram_tensor("xf", [128, NT * F], DT, kind="ExternalInput")
    cdeg_d = nc.dram_tensor("cdeg", [128, NT], f32, kind="ExternalInput")
    w1_d = nc.dram_tensor("w1", [F, F], DT, kind="ExternalInput")
    w2_d = nc.dram_tensor("w2", [F, F], DT, kind="ExternalInput")
    b1_d = nc.dram_tensor("b1", [1, F], DT, kind="ExternalInput")
    b2_d = nc.dram_tensor("b2", [1, F], DT, kind="ExternalInput")
    z_d = nc.dram_tensor("z", [128, 2 * F], f32, kind="ExternalOutput")

    ones_col_d = nc.inline_tensor(np.ones((128, 1), npdt), "ones_col")
    ones11_d = nc.inline_tensor(np.ones((1, 1), np.float32), "ones11")

    rg = [list(range(NCORES))]

    with tile.TileContext(nc) as tc:
        with (
            tc.tile_pool(name="big", bufs=1) as big,
            tc.tile_pool(name="work", bufs=2) as work,
            tc.tile_pool(name="ps", bufs=1, space="PSUM") as ps,
            tc.tile_pool(name="dram", bufs=1, space="DRAM") as dram,
        ):
            # ---- loads ----
            # scalar issues nothing: DMA issues serialize with ACT compute
            # on the same engine and would stall the sigmoid chain.
            onec = big.tile([128, 1], DT, name="onec_sb")
            nc.gpsimd.dma_start(onec[:], ones_col_d.ap())
            ones11 = big.tile([1, 1], f32, name="ones11_sb")
            nc.gpsimd.dma_start(ones11[:], ones11_d.ap())
            cdeg = big.tile([128, NT], f32, name="cdeg_sb")
            nc.gpsimd.dma_start(cdeg[:], cdeg_d.ap())
            myt = big.tile([128, 8 * HALF], FP8, name="myt_sb")
            for j in range(4):
                nc.sync.dma_start(myt[:, 2 * HALF * j:2 * HALF * (j + 1)],
                                  myt_d.ap()[:, 2 * HALF * j:2 * HALF * (j + 1)])
            myb = big.tile([128, 8 * F], DT, name="my_sb")
            nc.gpsimd.dma_start(myb[:], my_d.ap())
            adj = big.tile([128, NT * CPC], DT, name="adj_sb")
            # tiles 8-15 first (degree matmuls need them before the
            # sigmoid-gated tiles 0-7)
            nc.gpsimd.dma_start(adj[:, 8 * CPC:], adj_d.ap()[:, 8 * CPC:])
            nc.gpsimd.dma_start(adj[:, :8 * CPC], adj_d.ap()[:, :8 * CPC])
            xf = big.tile([128, NT * F], DT, name="xf_sb")
            nc.gpsimd.dma_start(xf[:], xf_d.ap())
            w1s = big.tile([F, F], DT, name="w1_sb")
            nc.gpsimd.dma_start(w1s[:], w1_d.ap())
            w2s = big.tile([F, F], DT, name="w2_sb")
            nc.gpsimd.dma_start(w2s[:], w2_d.ap())
            b1s = big.tile([1, F], DT, name="b1_sb")
            nc.gpsimd.dma_start(b1s[:], b1_d.ap())
            b2s = big.tile([1, F], DT, name="b2_sb")
            nc.gpsimd.dma_start(b2s[:], b2_d.ap())

            # ---- full deg_sig: sigmoid(masked_y^T) + free-dim reduce ----
            # degsig[p, j] = sum_s sigmoid(masked_y[s, 128j+p])
            degsig = big.tile([128, 8], f32, name="degsig_sb")
            for j in range(4):
                sgt = work.tile([128, 2 * HALF], DT, tag="sgt")
                nc.scalar.activation(sgt[:], myt[:, 2 * HALF * j:2 * HALF * (j + 1)],
                                     AF.Sigmoid)
                for u in range(2):
                    nc.vector.tensor_reduce(
                        degsig[:, 2 * j + u:2 * j + u + 1],
                        sgt[:, HALF * u:HALF * (u + 1)], axis=AX.X, op=ADD)

            # ---- Wgt dense block: adj += sigmoid(masked_y own-col shard) ----
            sg = work.tile([128, 8 * F], DT, tag="sg")
            nc.scalar.activation(sg[:], myb[:], AF.Sigmoid)
            for i in range(8):
                nc.vector.tensor_add(
                    adj[:, CPC * i:CPC * i + F], adj[:, CPC * i:CPC * i + F],
                    sg[:, F * i:F * (i + 1)])

            # ---- full dinv in per-partition layout [128, 16] ----
            dfull = big.tile([128, 8], f32, name="dfull_sb")
            nc.vector.tensor_tensor(dfull[:], cdeg[:, 0:8], degsig[:], op=ADD)
            sq_bc = big.tile([128, NT], f32, name="sqbc_sb")
            nc.scalar.activation(sq_bc[:, 0:8], dfull[:], AF.Sqrt)
            nc.scalar.activation(sq_bc[:, 8:16], cdeg[:, 8:16], AF.Sqrt)
            dinv_bc = big.tile([128, NT], f32, name="dinvbc_sb")
            nc.vector.reciprocal(dinv_bc[:], sq_bc[:])

            # ---- own-column degree row (for bias trick + dco) ----
            # colsums of the full Wgt shard; tiles 8-15 first (no sigmoid dep)
            ps_deg = ps.tile([1, CPC], f32, name="ps_deg")
            t_order = list(range(8, NT)) + list(range(8))
            for i, t in enumerate(t_order):
                nc.tensor.matmul(
                    ps_deg[:], onec[:], adj[:, CPC * t:CPC * (t + 1)],
                    start=(i == 0), stop=(i == NT - 1),
                )
            sqd = big.tile([1, CPC], f32, name="sqd_sb")     # sqrt(deg) (own)
            nc.scalar.activation(sqd[:], ps_deg[:], AF.Sqrt)
            sqb = big.tile([1, CPC], DT, name="sqb_sb")      # bf16 for bias mm
            nc.vector.tensor_copy(sqb[:], sqd[:])
            # dco[p, h] = dinv[own node 128h+p] via tiny transpose matmuls
            ps_dc = ps.tile([128, 2], f32, name="ps_dc")
            for h in range(2):
                nc.tensor.matmul(ps_dc[:, h:h + 1],
                                 sqd[:, 128 * h:128 * (h + 1)],
                                 ones11[:], start=(h == 0), stop=(h == 1))
            dco = big.tile([128, 2], f32, name="dco_sb")
            nc.vector.reciprocal(dco[:], ps_dc[:])

            # ---- fold dinv[src] into adjacency rows: adj~ = dinv_s * Wgt ----
            for t in range(NT):
                nc.vector.tensor_scalar_mul(
                    adj[:, CPC * t:CPC * (t + 1)], adj[:, CPC * t:CPC * (t + 1)],
                    dinv_bc[:, t:t + 1],
                )
            # fp8 copy of adj~ for layer 2
            adj8 = big.tile([128, NT * CPC], PAY, name="adj8_sb")
            nc.vector.tensor_scalar_mul(adj8[:], adj[:], S_ADJ)
            dco8 = big.tile([128, 2], f32, name="dco8_sb")
            nc.vector.tensor_scalar_mul(dco8[:], dco[:], S_H)
            dcoz = big.tile([128, 2], f32, name="dcoz_sb")
            nc.vector.tensor_scalar_mul(dcoz[:], dco[:], 1.0 / S_L2)
            sqbl2 = big.tile([1, CPC], DT, name="sqbl2_sb")
            nc.vector.tensor_scalar_mul(sqbl2[:], sqd[:], S_L2)

            def layer(xtiles, adjt, wsb, bsb, sqrow, name):
                # aggT[f, d] = sum_s x[s, f] * adj~[s, d]
                ps_agg = ps.tile([128, CPC], f32, name=f"ps_agg_{name}",
                                 tag="ps_agg")
                for t in range(NT):
                    nc.tensor.matmul(
                        ps_agg[:], xtiles[:, F * t:F * (t + 1)],
                        adjt[:, CPC * t:CPC * (t + 1)],
                        start=(t == 0), stop=(t == NT - 1),
                    )
                # r[d, o] = sum_f aggT[f, d]*W[f, o] + sqrt(deg_d)*b[o]
                # (agg chunk as stationary -> [dst, feat] directly); the
                # PSUM->SBUF cast is split per half so the first half's W
                # matmul overlaps the second half's cast.
                aggs = work.tile([128, CPC], DT, tag="aggs")
                ps_r = ps.tile([128, CPC], f32, name=f"ps_r_{name}", tag="ps_r")
                for h in range(2):
                    nc.vector.tensor_copy(aggs[:, 128 * h:128 * (h + 1)],
                                          ps_agg[:, 128 * h:128 * (h + 1)])
                    nc.tensor.matmul(ps_r[:, 128 * h:128 * (h + 1)],
                                     aggs[:, 128 * h:128 * (h + 1)], wsb[:],
                                     start=True, stop=False)
                    nc.tensor.matmul(ps_r[:, 128 * h:128 * (h + 1)],
                                     sqrow[:, 128 * h:128 * (h + 1)], bsb[:],
                                     start=False, stop=True)
                return ps_r

            # ---- layer 1 (fully local) ----
            ps_r1 = layer(xf, adj, w1s, b1s, sqb, "l1")
            # hidden[d,o] = relu(dinv_d*r) = dinv_d*relu(r)   (dinv > 0)
            g01 = work.tile([128, 2 * F], PAY, tag="g01")
            rl1 = work.tile([128, 2 * F], DT, tag="rl1")
            for h in range(2):
                nc.scalar.activation(rl1[:, 128 * h:128 * (h + 1)],
                                     ps_r1[:, 128 * h:128 * (h + 1)], AF.Relu)
                nc.vector.tensor_scalar_mul(
                    g01[:, 128 * h:128 * (h + 1)],
                    rl1[:, 128 * h:128 * (h + 1)], dco8[:, h:h + 1])

            # ---- the single collective: gather hidden [2048, 128] fp8 ----
            ag_in = dram.tile([CPC, F], PAY, name="ag_in")
            ag_out = dram.tile([N, F], PAY, name="ag_out", addr_space="Shared")
            nc.scalar.dma_start(ag_in[:].rearrange("(h p) c -> p h c", h=2), g01[:])
            nc.gpsimd.collective_compute(
                "AllGather", mybir.AluOpType.bypass,
                replica_groups=rg, ins=[ag_in.opt()], outs=[ag_out.opt()],
            )

            # gathered row 256k+128h+p = hidden[node 1024h+128k+p]; src tile
            # t=8h+k is a contiguous [128,128] block -> 3 parallel reloads
            xb2 = big.tile([128, NT * F], PAY, name="xb2_sb")
            v = ag_out[:].rearrange("(k h p) c -> h p k c", k=8, h=2)
            # one issue per queue, smallest first so matmul t=0 starts
            # while the rest still streams
            nc.scalar.dma_start(xb2[:, 0:2 * F], v[0][:, 0:2])
            nc.sync.dma_start(xb2[:, 2 * F:8 * F], v[0][:, 2:8])
            nc.gpsimd.dma_start(xb2[:, 8 * F:16 * F], v[1])

            # ---- layer 2 ----
            ps_r2 = layer(xb2, adj8, w2s, b2s, sqbl2, "l2")
            zt = work.tile([128, 2 * F], f32, tag="zt")
            zq = [nc.scalar, nc.sync]
            for h in range(2):
                nc.vector.tensor_scalar_mul(
                    zt[:, 128 * h:128 * (h + 1)],
                    ps_r2[:, 128 * h:128 * (h + 1)], dcoz[:, h:h + 1])
                zq[h].dma_start(z_d.ap()[:, 128 * h:128 * (h + 1)],
                                zt[:, 128 * h:128 * (h + 1)])

    nc.compile()
    return nc


def _host_prep(x, masked_y, W1, b1, Wmu, bmu, Wls, bls, edge_index, use_bf16):
    npdt = _np_dt(use_bf16)
    src = edge_index[0].astype(np.int64)
    dst = edge_index[1].astype(np.int64)

    A = np.zeros((N, N), np.float32)
    np.add.at(A, (src, dst), 1.0)
    idx = np.arange(N)
    A[idx, idx] += 1.0

    # integer degree counts (edges + self loop), replicated
    deg_cnt = np.bincount(dst, minlength=N).astype(np.float32) + 1.0
    cdeg_sw = np.ascontiguousarray(
        deg_cnt.reshape(NT, 128).T).astype(np.float32)

    # full masked_y transposed: myt[p, 1024j + s] = masked_y[s, 128j + p]
    # fp8 is plenty for the degree path: sigmoid errors average out over the
    # 1024-term column sums (the adjacency dense block stays bf16)
    import ml_dtypes
    M = masked_y[:HALF, :HALF]
    myt_sw = np.ascontiguousarray(
        M.T.reshape(8, 128, HALF).transpose(1, 0, 2).reshape(128, 8 * HALF)
    ).astype(ml_dtypes.float8_e4m3)

    # full x, global src tiles
    xf_sw = np.ascontiguousarray(
        x.reshape(NT, 128, F).transpose(1, 0, 2).reshape(128, NT * F)
    ).astype(npdt)

    W2 = np.concatenate([Wmu, Wls], axis=1).astype(npdt)
    b1r = np.ascontiguousarray(b1.reshape(1, F)).astype(npdt)
    b2r = np.concatenate([bmu, bls]).reshape(1, F).astype(npdt)
    W1c = np.ascontiguousarray(W1).astype(npdt)

    in_maps = []
    for k in range(NCORES):
        cols = np.r_[128 * k:128 * k + 128, HALF + 128 * k:HALF + 128 * k + 128]
        adj_k = A[:, cols]  # [2048, 256]
        adj_sw = np.ascontiguousarray(
            adj_k.reshape(NT, 128, CPC).transpose(1, 0, 2).reshape(128, NT * CPC)
        ).astype(npdt)
        my_k = masked_y[:HALF, F * k:F * (k + 1)]  # [1024, 128]
        my_sw = np.ascontiguousarray(
            my_k.reshape(8, 128, F).transpose(1, 0, 2).reshape(128, 8 * F)
        ).astype(npdt)
        in_maps.append({
            "adj": adj_sw,
            "my": my_sw,
            "myt": myt_sw,
            "xf": xf_sw,
            "cdeg": cdeg_sw,
            "w1": W1c,
            "w2": W2,
            "b1": b1r,
            "b2": b2r,
        })
    return in_maps


def _assemble(results):
    zfull = np.empty((N, F), np.float32)
    for k in range(NCORES):
        zk = results[k]["z"]  # [128, 256]: [own-node-in-block p, (h, feat)]
        zfull[128 * k:128 * (k + 1)] = zk[:, 0:128]
        zfull[HALF + 128 * k:HALF + 128 * (k + 1)] = zk[:, 128:256]
    return zfull[:, :F // 2].copy(), zfull[:, F // 2:].copy()


def _make_runner(nc):
    """Cached shard_map runner (mirror of bass2jax.run_bass_via_pjrt's
    multi-core branch, minus donation so the jitted fn is reusable)."""
    import jax
    from jax.sharding import Mesh, PartitionSpec
    from jax.experimental.shard_map import shard_map
    from concourse import bass2jax, mybir

    bass2jax.install_neuronx_cc_hook()

    def run(in_maps):
        from concourse import bass2jax as b2j
        results = b2j.run_bass_via_pjrt(nc, in_maps, n_cores=NCORES)
        return results

    return run


def kernel(x, masked_y, W1, b1, Wmu, bmu, Wls, bls, edge_index,
           _trace=False, _warm=True):
    use_bf16 = USE_BF16
    if "nc" not in _COMPILED or _COMPILED.get("bf16") != use_bf16:
        _COMPILED["nc"] = _build_program(use_bf16)
        _COMPILED["bf16"] = use_bf16
        _COMPILED["run"] = _make_runner(_COMPILED["nc"])

    in_maps = _host_prep(
        np.asarray(x, np.float32), np.asarray(masked_y, np.float32),
        np.asarray(W1, np.float32), np.asarray(b1, np.float32),
        np.asarray(Wmu, np.float32), np.asarray(bmu, np.float32),
        np.asarray(Wls, np.float32), np.asarray(bls, np.float32),
        np.asarray(edge_index), use_bf16,
    )
    run = _COMPILED["run"]
    if _warm and not _COMPILED.get("warmed"):
        run(in_maps)  # first call pays NEFF load on every core
        _COMPILED["warmed"] = True
    if _trace:
        import tempfile
        try:
            from antenv import axon_hooks
            hook = axon_hooks.get_axon_ntff_profile_hook()
        except ImportError:
            hook = None
        if hook is None:
            results = run(in_maps)
        else:
            neff_dir = tempfile.mkdtemp()
            with hook(neff_dir, list(range(NCORES))):
                results = run(in_maps)
            _COMPILED["ntff_dir"] = neff_dir
            try:
                import gauge.profiler
                from concourse._compat import FishPath
                from concourse.bass_utils import _process_ntff_profile
                profile = gauge.profiler.Profile(
                    profile_path=FishPath(neff_dir), kernel_dev_mode=True,
                    profile_on_exit=False, bass_kernel=_COMPILED["nc"].m,
                    offline_processing=True, fname="*_body*",
                )
                r = _process_ntff_profile(
                    profile, neff_dir, _COMPILED["nc"], list(range(NCORES)),
                    list(range(NCORES)), False, {}, trace_events=False,
                )
                _COMPILED["exec_time_ns"] = r.exec_time_ns
                _COMPILED["mean_exec_time_ns"] = r.mean_exec_time_ns
            except Exception as e:
                _COMPILED["exec_time_ns"] = None
                _COMPILED["trace_err"] = repr(e)
    else:
        results = run(in_maps)
    return _assemble(results)
